# revision 61
# baseline (speedup 1.0000x reference)
"""Trainium2 Bass kernel for a single causal attention head.

  x:  [32, 1024, 768] f32, Wq/Wk/Wv: [64, 768] f32
  out[b,q,:] = softmax_k(causal(Q K^T / 8)) @ V,  Q = x Wq^T etc.

Sharding: data-parallel over batch — 4 batches per core on 8 cores,
weights replicated.

The whole kernel runs in bf16 (measured end-to-end rel err ~6e-3 vs the
fp32 reference, threshold 2e-2). The host casts x to bf16 and pre-packs
the weight stationaries, which buys:

  1. Half the HBM traffic for x, and the transposition of x (contraction
     dim c must sit on partitions for the projection matmuls) rides the
     DMA xbar transpose engine (16-bit only) instead of costing ~190
     LDWEIGHTS+MATMUL pairs on the PE like the f32r version did. The
     xbar's issue rate (~1.34us per [1024,128] chunk, engine-blocking)
     is the scarce resource. All 24 chunks stream on the sync hwdge
     queue: splitting across sync+scalar corrupts data, because the tile
     scheduler (not emission order) fixes the global order that the 8
     DMAHW completion-semaphore lanes rotate over, so cross-queue lane
     sharing (out-of-order completion vs threshold waits) cannot be
     prevented from kernel code.
  2. bf16 matmul moving operands stream at 1 cycle/row at ANY width
     (f32r needs >=256 cols), so the short causal attention segments
     stop paying a 4x penalty, and LDWEIGHTS gets the fast-weight-load
     path.

per batch b (all on one NeuronCore):
  - xT [c=128, chunk j, t=1024] via 6 DMA xbar transposes.
  - two packed projection passes per 512-col half:
    [Wq^T|Wk^T] -> [Q^T; K^T] and [Wv^T|Wq^T] -> [V^T; Q^T] (the Q^T
    copy lands on partitions 64:128 where the S^T matmul needs its
    moving operand).
  - S^T = K^T.T Q^T per 128-row k-block (causal blocks only), additive
    -1e9 mask on the diagonal block (DVE), exp on the scalar engine.
  - AV in q-major orientation: stationary = E k-block chunk, moving =
    [V | ones] tile, accumulating out[q-tile, 65] directly in PSUM. The
    ones column makes col 64 the softmax denominator, and q-major means
    no output transposes or PSUM->SBUF copy chain — DVE normalizes
    straight out of PSUM.

Batches are software-pipelined: attention of batch b-1 interleaves with
the projection chunks of batch b so the PE never idles long enough for
the HAM clock gate to re-throttle; junk matmuls warm the clock during
the initial DMA fill.
"""

import os
import sys
import numpy as np

B_FULL = 32
N_CORES = 8
B_CORE = B_FULL // N_CORES
T, C, D = 1024, 768, 64
TT = T // 128  # 8
CC = C // 128  # 6
SCALE = 1.0 / np.sqrt(D)

_cache = {}


def _seg512(q0, q1):
    """Split [q0, q1) at 512 boundaries (PSUM bank limit)."""
    segs = []
    while q0 < q1:
        q_end = min(q1, (q0 // 512 + 1) * 512)
        segs.append((q0, q_end))
        q0 = q_end
    return segs


def _build():
    from contextlib import ExitStack

    import concourse.bass as bass
    import concourse.tile as tile
    from concourse import bacc, mybir
    from concourse.bass import ts
    from concourse.masks import make_causal_mask, make_identity

    f32 = mybir.dt.float32
    bf = mybir.dt.bfloat16
    nc = bacc.Bacc("TRN2", target_bir_lowering=False, debug=False)
    x = nc.dram_tensor("x", [B_CORE, T, C], bf, kind="ExternalInput").ap()
    w3 = nc.dram_tensor("w3", [128, CC, 192], bf, kind="ExternalInput").ap()
    y = nc.dram_tensor("y", [B_CORE, T, D], f32, kind="ExternalOutput").ap()

    with tile.TileContext(nc) as tc, ExitStack() as ctx:
        const = ctx.enter_context(tc.tile_pool(name="const", bufs=1))
        xtp = ctx.enter_context(tc.tile_pool(name="xt", bufs=4))
        sb = ctx.enter_context(tc.tile_pool(name="sb", bufs=2))
        epool = ctx.enter_context(tc.tile_pool(name="e", bufs=6))
        ypool = ctx.enter_context(tc.tile_pool(name="yout", bufs=4))
        # PSUM: 8 banks of [128 x 2KB]:
        #   ps_proj: 2 x [128,512] f32 (qk / vq half accums)     = 2 banks
        #   ps_st:   2 x [128,512] f32 (S^T chunks)              = 2 banks
        #   ps_vp:   2 x [128,4,65] bf16 ([V|1] tile transposes) = 2 banks
        #   ps_out:  2 x [128,4,65] f32 (q-major AV accum)       = 2 banks
        ps_proj = ctx.enter_context(tc.tile_pool(name="ps_proj", bufs=2, space="PSUM"))
        ps_st = ctx.enter_context(tc.tile_pool(name="ps_st", bufs=2, space="PSUM"))
        ps_vp = ctx.enter_context(tc.tile_pool(name="ps_vp", bufs=2, space="PSUM"))
        ps_out = ctx.enter_context(tc.tile_pool(name="ps_out", bufs=2, space="PSUM"))

        # ---- constants ----
        # single packed stationary [Wv^T|Wq^T|Wk^T]: WVQ = cols 0:128,
        # WQK = cols 64:192 (overlapping views share the Wq^T middle), so
        # only ONE weight DMA sits ahead of the transpose stream
        W3 = const.tile([128, CC, 192], bf, tag="w3")
        nc.sync.dma_start(W3, w3)
        WQK = W3[:, :, 64:192]
        WVQ = W3[:, :, 0:128]
        ident = const.tile([128, 128], f32, tag="ident")
        make_identity(nc, ident)
        ident_b = const.tile([128, 128], bf, tag="ident_b")
        nc.gpsimd.tensor_copy(ident_b, ident)
        # additive causal mask for the S^T diagonal block: 0 where k<=q
        # (p<=f), -1e9 where k>q
        dmask = const.tile([128, 128], f32, tag="dmask")
        nc.gpsimd.memset(dmask, 0.0)
        nc.gpsimd.affine_select(
            out=dmask,
            in_=dmask,
            compare_op=mybir.AluOpType.is_ge,
            fill=-1e9,
            base=0,
            pattern=[[1, 128]],
            channel_multiplier=-1,
        )

        # upper-triangular -1e9 mask as a PE stationary: umask.T @ I
        # accumulates the same additive mask as dmask, keeping the tail's
        # diagonal blocks on a PE->PE->ACT chain (no DVE hop)
        umask = const.tile([128, 128], f32, tag="umask")
        make_causal_mask(nc, umask, mask_val=-1e9)
        umask_b = const.tile([128, 128], bf, tag="umask_b")
        nc.gpsimd.tensor_copy(umask_b, umask)

        states = {b: {"b": b, "E": {}} for b in range(B_CORE)}

        def op_xt(b, j, eng):
            """DMA xbar transpose of one [1024,128] c-chunk of x[b]."""
            st8 = states[b]
            if "xT" not in st8:
                st8["xT"] = xtp.tile([128, CC, T], bf, tag="xT", name=f"xT{b}")
            eng.dma_start(st8["xT"][:, j, :], x[b][:, ts(j, 128)], transpose=True)

        def op_proj_h0_j(b, j):
            st8 = states[b]
            if j == 0:
                st8["QK_sb"] = sb.tile([128, T], bf, tag="qksb", name="QK_sb")
                st8["Qd_sb"] = sb.tile([128, T], bf, tag="qd", name="Qd_sb")
                st8["VT_sb"] = sb.tile([128, T], bf, tag="vtsb", name="VT_sb")
                st8["qk_h0"] = ps_proj.tile([128, 512], f32, tag="ps_proj", name="qk_h0")
                st8["vq_h0"] = ps_proj.tile([128, 512], f32, tag="ps_proj", name="vq_h0")
            for W, ps in ((WQK, st8["qk_h0"]), (WVQ, st8["vq_h0"])):
                nc.tensor.matmul(
                    ps,
                    W[:, j, :],
                    st8["xT"][:, j, 0:512],
                    start=(j == 0),
                    stop=(j == CC - 1),
                )

        def _stage_half(st8, h, qk_ps, vq_ps):
            hs = ts(h, 512)
            nc.vector.tensor_copy(st8["QK_sb"][:, hs], qk_ps)
            # Q^T dup: psum rows 64:128 -> SBUF rows 64:128 (same partitions)
            nc.vector.tensor_copy(st8["Qd_sb"][64:128, hs], vq_ps[64:128, :])
            nc.vector.tensor_copy(st8["VT_sb"][0:64, hs], vq_ps[0:64, :])

        def op_proj_h0_stage(b):
            st8 = states[b]
            _stage_half(st8, 0, st8.pop("qk_h0"), st8.pop("vq_h0"))

        def op_proj_h1(b):
            st8 = states[b]
            qk_ps = ps_proj.tile([128, 512], f32, tag="ps_proj", name="qk_ps")
            vq_ps = ps_proj.tile([128, 512], f32, tag="ps_proj", name="vq_ps")
            for W, ps in ((WQK, qk_ps), (WVQ, vq_ps)):
                for j in range(CC):
                    nc.tensor.matmul(
                        ps,
                        W[:, j, :],
                        st8["xT"][:, j, 512:1024],
                        start=(j == 0),
                        stop=(j == CC - 1),
                    )
            st8.pop("xT")
            _stage_half(st8, 1, qk_ps, vq_ps)
            # ones row for the softmax denominator column
            nc.gpsimd.memset(st8["VT_sb"][64:65, :], 1.0)

        def op_vp(b):
            """[V | ones] k-major blocks: Vp[p, kt, :] = [V[kt*128+p, :] 1]."""
            st8 = states[b]
            VT_sb = st8["VT_sb"]
            # D+2 stride keeps per-tile byte offsets 4-aligned (PSUM req)
            Vp = sb.tile([128, TT, D + 2], bf, tag="vp", name=f"Vp{b}")
            for g in range(2):
                # full-bank tile (2048B): sub-bank PSUM tiles can share a
                # bank with the AV accumulators, and a start=True matmul
                # clears its whole bank on HW
                pv = ps_vp.tile([128, 4, 256], bf, tag="ps_vp", name="pv")
                for u in range(4):
                    k_i = g * 4 + u
                    nc.tensor.transpose(
                        pv[:, u, 0 : D + 1],
                        VT_sb[0:65, ts(k_i, 128)],
                        ident_b[0:65, 0:65],
                    )
                nc.vector.tensor_copy(
                    Vp[:, g * 4 : (g + 1) * 4, 0 : D + 1], pv[:, :, 0 : D + 1]
                )
            st8["Vp"] = Vp

        def op_oalloc(b):
            # [128, 4, 128] f32 = exactly one 2KB bank per tile
            oA = ps_out.tile([128, 4, 128], f32, tag="ps_out", name="oA")
            oB = ps_out.tile([128, 4, 128], f32, tag="ps_out", name="oB")
            states[b]["o_AB"] = (oA, oB)

        def _sT_segs(b, kt, segs, pe_mask=False):
            st8 = states[b]
            QK_sb, Qd_sb = st8["QK_sb"], st8["Qd_sb"]
            if kt not in st8["E"]:
                st8["E"][kt] = epool.tile([128, T], bf, tag="e", name=f"E_{b}_{kt}")
            E = st8["E"][kt]
            for (q0, q1) in segs:
                st_c = ps_st.tile([128, 512], f32, tag="ps_st", name="st_c")
                n = q1 - q0
                diag = q0 == kt * 128
                nc.tensor.matmul(
                    st_c[:, 0:n],
                    QK_sb[64:128, ts(kt, 128)],
                    Qd_sb[64:128, q0:q1],
                    start=True,
                    stop=not (diag and pe_mask),
                )
                if diag and pe_mask:
                    nc.tensor.matmul(
                        st_c[:, 0:128], umask_b, ident_b, start=False, stop=True
                    )
                elif diag:
                    nc.vector.tensor_add(st_c[:, 0:128], st_c[:, 0:128], dmask)
                nc.scalar.activation(
                    E[:, q0:q1],
                    st_c[:, 0:n],
                    mybir.ActivationFunctionType.Exp,
                    scale=float(SCALE),
                )

        def op_sT(b, kt, pe_mask=False):
            """S^T 512-col chunks for k-block kt + diagonal mask + exp."""
            _sT_segs(b, kt, _seg512(kt * 128, T), pe_mask)

        def op_sT1(b, kt):
            """First S^T segment (q < 512) for kt < 4: depends only on the
            h0 halves of QK/Qd, so it can run right after this batch's own
            projection — pulling the serial scalar-engine exp chain ~2
            segments earlier per batch."""
            _sT_segs(b, kt, [(kt * 128, 512)])

        def op_sT2(b, kt):
            """Remaining S^T segment (512 <= q < 1024) for kt < 4."""
            _sT_segs(b, kt, [(512, T)])

        def op_av(b, kt, fine=False):
            """q-major AV: stationary = E chunk, accumulate out[q-tile, 65].
            fine=True (tail batch): region qt is final after AV(kt=qt), and
            the normalization reads it before the bank's accumulation group
            formally closes — legal on HW (per-region has_written), but the
            sim's group model needs the check skipped."""
            st8 = states[b]
            (oA, oB), Vp = st8["o_AB"], st8["Vp"]
            E = st8["E"].pop(kt)
            for qt in range(kt, TT):
                o = oA if qt < 4 else oB
                nc.tensor.matmul(
                    o[:, qt % 4, 0 : D + 1],
                    E[:, ts(qt, 128)],
                    Vp[:, kt, 0 : D + 1],
                    start=(kt == 0 and qt in (0, 4)),
                    stop=(kt == 3 and qt == 3) or (kt == 7 and qt == 7),
                    skip_group_check=fine,
                )

        def op_out_q(b, qt):
            """Normalize one q-tile straight out of PSUM."""
            st8 = states[b]
            if qt == 0:
                st8["y_sb"] = ypool.tile([128, TT, D], f32, tag="y", name="y_sb")
            oA, oB = st8["o_AB"]
            o = oA if qt < 4 else oB
            rec = sb.tile([128, 1], f32, tag="rec")
            nc.vector.reciprocal(rec, o[:, qt % 4, D : D + 1])
            nc.vector.tensor_scalar_mul(st8["y_sb"][:, qt, :], o[:, qt % 4, 0:D], rec)

        def op_out_dma(b, g):
            st8 = states[b]
            nc.gpsimd.dma_start(
                y[b].rearrange("(t p) d -> p t d", p=128)[:, 4 * g : 4 * (g + 1), :],
                st8["y_sb"][:, 4 * g : 4 * (g + 1), :],
            )
            if g == 1:
                st8.pop("o_AB")
                st8.pop("y_sb")

        # ---- pipeline schedule ----
        # steady j-slot mapping: S^T(kt) in slot JS, its AV ~2 slots later
        # (projection chunks hide the exp latency)
        # slot map: seg-2s of kt 0/1 first (their seg-1s ran at the END
        # of the previous step, right after this batch's own projections,
        # so the serial exp chain starts ~2 segments earlier per batch)
        JS = {0: [(op_sT2, 0), (op_sT2, 1)], 1: [(op_sT, 2)], 2: [(op_sT, 3)],
              3: [(op_sT, 4)], 4: [(op_sT, 5)], 5: [(op_sT, 6), (op_sT, 7)]}
        JA = {1: [0], 2: [1], 3: [2], 4: [3, 4], 5: [5]}

        def steady_step(front_b, att_b):
            for j in range(CC):
                if att_b is not None:
                    for fn, kt in JS.get(j, []):
                        fn(att_b, kt)
                if front_b is not None:
                    op_proj_h0_j(front_b, j)
                if att_b is not None:
                    for kt in JA.get(j, []):
                        op_av(att_b, kt)
            if front_b is not None:
                op_proj_h0_stage(front_b)
                op_proj_h1(front_b)
                op_sT1(front_b, 0)
                op_sT1(front_b, 1)
            if att_b is not None:
                op_av(att_b, 6)
                op_av(att_b, 7)
                for qt in range(4):
                    op_out_q(att_b, qt)
            if front_b is not None:
                op_vp(front_b)
                op_oalloc(front_b)
            if att_b is not None:
                for qt in range(4, TT):
                    op_out_q(att_b, qt)

        # xT transposes: all on the sync hwdge queue (see docstring for
        # why dual-queue is unsafe); weights were emitted ahead of these
        for b in (0, 1):
            for j in range(CC):
                op_xt(b, j, nc.sync)
        for j in range(CC):
            op_xt(2, j, nc.sync)

        # warm-keeper junk matmuls: PE is idle until xT(0) lands (~6us);
        # keep the HAM activity monitor fed so proj(0) starts at 2.4GHz.
        wflat = W3.rearrange("p a b -> p (a b)")
        junk = ps_st.tile([128, 512], f32, tag="ps_st", name="junk")
        for _ in range(4):
            nc.tensor.matmul(junk, WQK[:, 0, :], wflat[:, 0:512], start=True, stop=True)

        # step 0: front(0) only
        steady_step(0, None)
        op_vp_done_marker = None  # noqa: F841
        # steps 1..3: front(s) + att(s-1); b3's transposes are emitted at
        # the head of step 2; y(b) DMA emissions are spread after the LAST
        # transpose emission so they drain during compute instead of
        # serializing on the gpsimd ring at the end
        for s in (1, 2, 3):
            if s == 2:
                for j in range(CC):
                    op_xt(3, j, nc.sync)
            if s == 3:
                op_out_dma(0, 0)
                op_out_dma(0, 1)
            steady_step(s, s - 1)
        # step 4: att(3) alone, lag-2 pipelined (seg-1s of kt 0/1 already
        # ran at the end of step 3)
        # Output region qt receives writes only from k-blocks <= qt, so
        # its normalization can chase AV(3, qt) immediately and y(3)'s
        # first half ships while the second half is still accumulating —
        # shortening the serial chain between the last exp and the exit
        # barrier.
        tail_ops = [(op_sT2, 0), (op_sT2, 1)] + [(op_sT, k) for k in range(2, TT)]
        for i, (fn, kt) in enumerate(tail_ops):
            if fn is op_sT:
                fn(3, kt, pe_mask=True)
            else:
                fn(3, kt)
            if i >= 2:
                op_av(3, i - 2, fine=True)
                op_out_q(3, i - 2)
        op_av(3, 6, fine=True)
        op_out_q(3, 6)
        op_out_dma(1, 0)
        op_out_dma(1, 1)
        op_out_dma(3, 0)
        op_av(3, 7, fine=True)
        op_out_q(3, 7)
        op_out_dma(2, 0)
        op_out_dma(2, 1)
        op_out_dma(3, 1)

    nc.compile()
    return nc


def _get_nc():
    if "nc" not in _cache:
        _cache["nc"] = _build()
    return _cache["nc"]


def prep_inputs(inputs):
    """Cast x to bf16 and pre-pack the projection stationaries.

    w3[p, j, :] = [Wv^T | Wq^T | Wk^T] chunk j — the on-device WVQ and
    WQK stationaries are the overlapping 128-wide views [0:128], [64:192].
    """
    import ml_dtypes

    bf16 = ml_dtypes.bfloat16
    x = np.asarray(inputs["x"]).astype(bf16)
    wq = np.asarray(inputs["Wq"], dtype=np.float32)
    wk = np.asarray(inputs["Wk"], dtype=np.float32)
    wv = np.asarray(inputs["Wv"], dtype=np.float32)
    w3 = np.empty((128, CC, 192), dtype=np.float32)
    for j in range(CC):
        cs = slice(j * 128, (j + 1) * 128)
        w3[:, j, 0:D] = wv[:, cs].T
        w3[:, j, D : 2 * D] = wq[:, cs].T
        w3[:, j, 2 * D : 3 * D] = wk[:, cs].T
    return x, w3.astype(bf16)


def run(inputs, trace=False, tmpdir=None):
    """Shard, run on 8 cores, gather. Returns (y_full, BassKernelResults)."""
    from concourse.bass_utils import run_bass_kernel_spmd

    x, w3 = prep_inputs(inputs)
    assert x.shape == (B_FULL, T, C)

    nc = _get_nc()
    in_maps = [
        {
            "x": np.ascontiguousarray(x[i * B_CORE : (i + 1) * B_CORE]),
            "w3": w3,
        }
        for i in range(N_CORES)
    ]
    kwargs = {}
    if trace:
        _install_trace_shim()
        kwargs = {"trace": True, "tmpdir": tmpdir}
    res = run_bass_kernel_spmd(nc, in_maps, list(range(N_CORES)), **kwargs)
    out = np.concatenate([res.results[i]["y"] for i in range(N_CORES)], axis=0)
    return out, res


def kernel(**inputs) -> np.ndarray:
    out, _ = run(inputs, trace=False)
    return out


def _install_trace_shim():
    """The image's antenv lacks axon_hooks; register the NTFF profile hook
    ourselves so run_bass_kernel_spmd(trace=True) works. Test-only path."""
    import types

    try:
        from antenv.axon_hooks import get_axon_ntff_profile_hook  # noqa: F401

        return
    except ImportError:
        pass
    import antenv
    from trn_agent_boot.trn_boot import _ntff_profile_via_ctypes

    mod = types.ModuleType("antenv.axon_hooks")
    mod._hook = _ntff_profile_via_ctypes("/opt/axon/libaxon_pjrt.so")
    mod.set_axon_ntff_profile_hook = lambda h: setattr(mod, "_hook", h)
    mod.get_axon_ntff_profile_hook = lambda: mod._hook
    sys.modules["antenv.axon_hooks"] = mod
    antenv.axon_hooks = mod

    import concourse.bass_utils as bu

    bu.upload_artifacts = lambda tmpdir: tmpdir
